# revision 49
# baseline (speedup 1.0000x reference)
"""Causal multi-head attention block (qkv proj + partial RoPE + causal attn +
out proj) for Trainium2, distributed over 8 NeuronCores.

Sharding: core i handles batch b = i//2 and head-group g = i%2 (6 of 12 heads).
Each core computes a partial output projection (contraction over its 6 heads'
384 channels); the host sums the two head-group partials per batch.

v9 design notes (v2 342us, v4 300us, v5 294us, v6 287us, v8 285us; v9
measured 319us vs v8's 336us under the same ~18% P0 thermal downclock,
i.e. ~270us at full clock -- engine-busy ratios v9/v8 are uniformly ~1.19,
confirming identical instruction mix at a lower clock with better overlap):
  - All-bf16 matmul paths. exp is the only Scalar-engine op.
  - Attention inner loop software-pipelined by one k-tile: the score pair +
    exp of kt+1 are emitted BEFORE the attn@v matmuls of kt, so the strict
    in-order PE queue never parks ready score work behind an av matmul
    that is waiting on its exp.
  - Pipeline: projection of tile jt+2 is EMITTED two iterations ahead
    (projections depend only on external inputs), so the projection chain
    (x DMA -> rope matmuls -> DVE rope -> scatter DMAs -> pass evictions)
    has a full iteration of slack: nothing in it can gate the exp stream
    or the attention matmuls, and the DVE eviction queue order stops
    mattering.  v5 measured the tile-0 scatter stream alone at ~15us.
  - Causal mask applied in one GpSimd op per diagonal k-tile over both
    heads.  (A 24->4 merged rope scatter via partition-split DMA views
    produced wrong data -- reverted to 24 small DMAs split across rings.)
  - Iteration emit order: attention(jt) | projection(jt+2) | finalize(jt-1)
    (finalize = softmax-normalize + out-projection, software-pipelined one
    tile behind; its serial chain fills engine gaps).
  - Scatter/bulk DMAs split across both HWDGE rings (sync + scalar) to
    double the small-transfer stream rate.
  - PSUM: sg scores [128,2,512]x2 (4 banks) + o accumulator x1 (2 banks) +
    merged flex pool [128,512]x2 (2 banks) shared by rope/pass/v projection
    tiles, broadcast, and out-projection.
  - Softmax: no max-subtraction needed (|scores/8| < ~3), denominator via a
    ones-column in v (o row 64); o evicted bf16 by DVE, rowsums DMA'd to
    rs6, DVE reciprocal, K=6 broadcast matmul, DVE multiply into bf16 o_sb.
"""

import numpy as np

B, T, C = 4, 2048, 768
NH, HD, RD = 12, 64, 16
NHL = NH // 2          # heads per core (local)
NPAIR = NHL // 2       # head pairs per core
CL = NHL * HD          # local channels (384)
TQ = 512               # q tile
NTQ = T // TQ
NKT = T // 128         # k tiles of 128

_cache = {}


def _build(debug=False):
    import concourse.bacc as bacc
    import concourse.mybir as mybir
    import concourse.tile as tile

    F32 = mybir.dt.float32
    BF16 = mybir.dt.bfloat16
    AF = mybir.ActivationFunctionType
    MUL = mybir.AluOpType.mult
    SUB = mybir.AluOpType.subtract
    ADD = mybir.AluOpType.add

    nc = bacc.Bacc(trn_type="TRN2", name="attn8")

    xt = nc.dram_tensor("xt", [C, T], BF16, kind="ExternalInput")
    wqkt = nc.dram_tensor("wqkt", [C, 2 * CL], BF16, kind="ExternalInput")
    wvt = nc.dram_tensor("wvt", [C, CL], BF16, kind="ExternalInput")
    wot = nc.dram_tensor("wot", [CL, C], BF16, kind="ExternalInput")
    cosb = nc.dram_tensor("cosb", [96, T], BF16, kind="ExternalInput")
    sinb = nc.dram_tensor("sinb", [96, T], BF16, kind="ExternalInput")
    tri = nc.dram_tensor("tri", [128, 256], BF16, kind="ExternalInput")
    e6 = nc.dram_tensor("e6", [6, NPAIR * 128], BF16, kind="ExternalInput")
    out = nc.dram_tensor("out", [C, T], BF16, kind="ExternalOutput")

    # qk-projection M-tiles (wqkt column order, host-built):
    #   tile 0: r1 rows [96] = (q h0..h5 | k h0..h5) x dims 0:8
    #   tile 1: r2 rows [96] = same x dims 8:16
    #   tiles 2..6: pass [128,128,128,128,64] = (q h0..h5 | k h0..h5) x dims 16:64
    MT_SIZES = [96, 96, 128, 128, 128, 128, 64]
    MT_OFF = np.cumsum([0] + MT_SIZES).tolist()

    def pass_dest(row):
        a, r = divmod(row, 48)        # a: tensor-head 0..11 (q first), r: dim-16
        tn, hl = divmod(a, NHL)
        blk = (0 if tn == 0 else NPAIR) + hl // 2
        part = 64 * (hl % 2) + 16 + r
        return blk, part

    with tile.TileContext(nc) as tc:
        with (
            tc.tile_pool(name="persist", bufs=1) as pp,
            tc.tile_pool(name="weights", bufs=1) as wp,
            tc.tile_pool(name="xload", bufs=3) as xlp,
            tc.tile_pool(name="pstage", bufs=3) as psg,
            tc.tile_pool(name="ropet", bufs=3) as rtp,
            tc.tile_pool(name="expp", bufs=6) as xpp,
            tc.tile_pool(name="misc", bufs=3) as msc,
            tc.tile_pool(name="onorm", bufs=8) as onp,
            tc.tile_pool(name="rsix", bufs=2) as rsp,
            tc.tile_pool(name="flex", bufs=2, space="PSUM") as flx,
            tc.tile_pool(name="sps", bufs=2, space="PSUM") as sps,
            tc.tile_pool(name="ops", bufs=1, space="PSUM") as ops,
        ):
            qk_sb = pp.tile([128, 2 * NPAIR, T], BF16, tag="qk")
            v_sb = pp.tile([128, NKT, NHL, HD + 1], BF16, tag="v")
            o_sb = pp.tile([128, NPAIR, T], BF16, tag="o")
            cos_t = pp.tile([96, T], BF16, tag="cos")
            sin_t = pp.tile([96, T], BF16, tag="sin")
            tri_t = pp.tile([128, 256], BF16, tag="tri")
            e6_t = pp.tile([6, NPAIR * 128], BF16, tag="e6")
            rot1 = pp.tile([96, T], BF16, tag="rot1")
            rot2 = pp.tile([96, T], BF16, tag="rot2")

            wqk_t = wp.tile([128, C // 128, 2 * CL], BF16, tag="wqk")
            wv_t = wp.tile([128, C // 128, CL], BF16, tag="wv")
            wo_t = wp.tile([128, NPAIR, C], BF16, tag="wo")

            # sync ring: wqk/wv/x + half the scatters (the critical path).
            # scalar ring: cos/sin/wo + other half + rs6 + out stores.
            nc.sync.dma_start(wqk_t, wqkt.rearrange("(co p) m -> p co m", p=128))
            nc.sync.dma_start(tri_t, tri[:, :])
            nc.sync.dma_start(e6_t, e6[:, :])
            nc.gpsimd.memset(
                v_sb[:, :, :, HD:HD + 1]
                .bitcast(mybir.dt.uint16)
                .rearrange("p a b c -> p (a b c)"),
                0x3F80)  # bf16 1.0 bit pattern
            # x(0) first on the scalar ring: the first qk matmuls need only
            # wqk (sync ring) + x(0); cos/sin/wv/wo are consumed later.
            x_pre = xlp.tile([128, C // 128, TQ], BF16, tag="x")
            nc.scalar.dma_start(
                x_pre, xt.rearrange("(co p) t -> p co t", p=128)[:, :, 0:TQ])
            nc.scalar.dma_start(cos_t, cosb[:, :])
            nc.scalar.dma_start(sin_t, sinb[:, :])
            nc.scalar.dma_start(wv_t, wvt.rearrange("(co p) m -> p co m", p=128))
            nc.scalar.dma_start(wo_t, wot.rearrange("(po p) m -> p po m", p=128))

            def _project(jt):
                ts = slice(jt * TQ, (jt + 1) * TQ)
                if jt == 0:
                    x_jt = x_pre
                else:
                    x_jt = xlp.tile([128, C // 128, TQ], BF16, tag="x")
                    nc.scalar.dma_start(
                        x_jt,
                        xt.rearrange("(co p) t -> p co t", p=128)[:, :, ts])

                psa = flx.tile([128, TQ], F32, tag="flex")
                psb = flx.tile([128, TQ], F32, tag="flex")
                for mt, psr in ((0, psa), (1, psb)):
                    R = MT_SIZES[mt]
                    for c in range(C // 128):
                        nc.tensor.matmul(
                            psr[0:R], wqk_t[:, c, MT_OFF[mt]:MT_OFF[mt + 1]],
                            x_jt[:, c], start=(c == 0), stop=(c == C // 128 - 1))
                # rope: rot1 = r1*cos - r2*sin ; rot2 = r2*cos + r1*sin
                t1 = rtp.tile([96, TQ], F32, tag="t1")
                t2 = rtp.tile([96, TQ], F32, tag="t2")
                nc.vector.tensor_tensor(t1, psa[0:96], cos_t[:, ts], MUL)
                nc.vector.tensor_tensor(t2, psb[0:96], sin_t[:, ts], MUL)
                nc.vector.tensor_tensor(rot1[:, ts], t1, t2, SUB)
                t3 = rtp.tile([96, TQ], F32, tag="t1")
                t4 = rtp.tile([96, TQ], F32, tag="t2")
                nc.vector.tensor_tensor(t3, psb[0:96], cos_t[:, ts], MUL)
                nc.vector.tensor_tensor(t4, psa[0:96], sin_t[:, ts], MUL)
                nc.vector.tensor_tensor(rot2[:, ts], t3, t4, ADD)
                # scatters 3-way split: both HWDGE rings + the gpsimd SWDGE
                # path as a third independent stream (~1 DMA/us each)
                rings = (nc.sync, nc.scalar, nc.gpsimd)
                # pair-first order: attention pair p needs q-block p (heads
                # 2p,2p+1 = a 2p,2p+1) and k-block (a 6+2p,7+2p) — emit
                # those first so pair 0's scores unblock after 8 scatters
                for a in (0, 1, 6, 7, 2, 3, 8, 9, 4, 5, 10, 11):
                    tn, hl = divmod(a, NHL)
                    blk = (0 if tn == 0 else NPAIR) + hl // 2
                    base = 64 * (hl % 2)
                    eng = rings[a % 3]
                    eng.dma_start(qk_sb[base:base + 8, blk, ts],
                                  rot1[8 * a:8 * a + 8, ts])
                    eng.dma_start(qk_sb[base + 8:base + 16, blk, ts],
                                  rot2[8 * a:8 * a + 8, ts])

                nsc = 0
                for mt in range(2, 7):
                    R = MT_SIZES[mt]
                    ps = flx.tile([128, TQ], F32, tag="flex")
                    for c in range(C // 128):
                        nc.tensor.matmul(
                            ps[0:R], wqk_t[:, c, MT_OFF[mt]:MT_OFF[mt + 1]],
                            x_jt[:, c], start=(c == 0), stop=(c == C // 128 - 1))
                    stg = psg.tile([128, TQ], BF16, tag="pstg")
                    nc.vector.tensor_copy(stg[0:R], ps[0:R])
                    row0 = MT_OFF[mt] - MT_OFF[2]
                    row = row0
                    while row < row0 + R:
                        blk, part = pass_dest(row)
                        run = min(row0 + R - row, 48 - row % 48)
                        eng = rings[nsc % 3]
                        nsc += 1
                        eng.dma_start(
                            qk_sb[part:part + run, blk, ts],
                            stg[row - row0:row - row0 + run])
                        row += run

                for vt in range(TQ // 128):
                    pvf = flx.tile([128, TQ], F32, tag="flex")
                    pv = pvf[:, 0:CL]
                    kt0 = jt * (TQ // 128) + vt
                    for c in range(C // 128):
                        nc.tensor.matmul(
                            pv, x_jt[:, c, vt * 128:(vt + 1) * 128],
                            wv_t[:, c], start=(c == 0), stop=(c == C // 128 - 1))
                    nc.vector.tensor_copy(
                        v_sb[:, kt0, :, 0:HD],
                        pv.rearrange("p (h d) -> p h d", d=HD))

            def _attention(jq):
                qs = slice(jq * TQ, (jq + 1) * TQ)
                ouns = []
                rs6_t = rsp.tile([6, TQ], BF16, tag="rs6")
                for p in range(NPAIR):
                    qb = qk_sb[:, p, qs]
                    kb = qk_sb[:, NPAIR + p, :]
                    o_ps = ops.tile([128, 2, TQ], F32, tag="o")
                    nkt = 4 * (jq + 1)
                    # scores/exp emitted one k-tile ahead of the attn@v
                    # matmuls: the PE queue is strict in-order, so putting
                    # av(kt) (which waits on exp(kt)) ahead of the ready
                    # score pair (kt+1) stalls the ready work behind it.
                    pend_av = None

                    def _av(item, last):
                        kt, a = item
                        for h in range(2):
                            nc.tensor.matmul(
                                o_ps[0:65, h, a:TQ],
                                v_sb[:, kt, 2 * p + h, :],
                                eps[kt][:, h, a:TQ],
                                start=(kt == 0), stop=last)

                    eps = {}
                    for kt in range(nkt):
                        m = kt - 4 * jq
                        a = 0 if m < 0 else 128 * m
                        ks = slice(kt * 128, (kt + 1) * 128)
                        sg = sps.tile([128, 2, TQ], F32, tag="s")
                        nc.tensor.matmul(
                            sg[:, 0, a:TQ], kb[0:64, ks], qb[0:64, a:TQ],
                            start=True, stop=True, tile_position=(0, 0))
                        nc.tensor.matmul(
                            sg[:, 1, a:TQ], kb[64:128, ks], qb[64:128, a:TQ],
                            start=True, stop=True, tile_position=(64, 0))
                        ep = xpp.tile([128, 2, TQ], BF16, tag="e")
                        nc.scalar.activation(ep[:, :, a:TQ], sg[:, :, a:TQ],
                                             AF.Exp, scale=0.125)
                        if m >= 0:
                            nc.gpsimd.tensor_tensor(
                                ep[:, :, a:a + 128],
                                ep[:, :, a:a + 128],
                                tri_t.rearrange("p (h w) -> p h w", h=2), MUL)
                        eps[kt] = ep
                        if pend_av is not None:
                            _av(pend_av, last=False)
                            del eps[pend_av[0]]
                        pend_av = (kt, a)
                    _av(pend_av, last=True)
                    # o -> SBUF unnormalized (DVE, bf16); rowsum row -> rs6
                    oun = onp.tile([128, 2, TQ], BF16, tag="oun")
                    nc.vector.tensor_copy(oun[0:65, :, :], o_ps[0:65, :, :])
                    ouns.append(oun)
                    for h in range(2):
                        nc.gpsimd.dma_start(
                            rs6_t[2 * p + h:2 * p + h + 1, :],
                            oun[64:65, h, :])
                return (jq, qs, ouns, rs6_t)

            def _finalize(fjt, qs, ouns, rs6_t):
                # batched softmax denominators for all 3 pairs of this q-tile
                rinv6_t = rsp.tile([6, TQ], BF16, tag="rinv6")
                with nc.allow_low_precision(reason="softmax denom in bf16"):
                    nc.vector.reciprocal(rinv6_t, rs6_t)
                for p in range(NPAIR):
                    bc = flx.tile([128, TQ], F32, tag="flex")
                    nc.tensor.matmul(bc, e6_t[:, p * 128:(p + 1) * 128],
                                     rinv6_t, start=True, stop=True)
                    oun = ouns[p]
                    nc.vector.tensor_tensor(
                        o_sb[0:64, p, qs], oun[0:64, 0, :], bc[0:64], MUL)
                    nc.vector.tensor_tensor(
                        o_sb[64:128, p, qs], oun[0:64, 1, :], bc[64:128], MUL)
                # output projection for this t-tile
                for dt in range(C // 128):
                    po = flx.tile([128, TQ], F32, tag="flex")
                    for p in range(NPAIR):
                        nc.tensor.matmul(
                            po, wo_t[:, p, dt * 128:(dt + 1) * 128],
                            o_sb[:, p, qs], start=(p == 0), stop=(p == NPAIR - 1))
                    ost = msc.tile([128, TQ], BF16, tag="ost")
                    nc.vector.tensor_copy(ost, po)
                    nc.scalar.dma_start(
                        out.rearrange("(do p) t -> do p t", p=128)[dt, :, qs], ost)

            _project(0)
            pending = []
            for jt in range(NTQ):
                pending.append(_attention(jt))
                if jt == 0:
                    _project(1)   # after att(0) so attention outranks it
                if jt + 2 < NTQ:
                    _project(jt + 2)
                if jt > 0:
                    _finalize(*pending.pop(0))
            _finalize(*pending.pop(0))

    nc.compile()
    return nc


def _host_inputs(x, w_qkv, w_out):
    """Build per-core input dicts. Core i: batch i//2, head-group i%2."""
    import ml_dtypes

    BF = ml_dtypes.bfloat16
    xf = np.ascontiguousarray(x, dtype=np.float32)
    w3 = np.asarray(w_qkv, dtype=np.float32).reshape(3, NH, HD, C)
    wo = np.asarray(w_out, dtype=np.float32)

    per_group = []
    for g in range(2):
        hs = range(g * NHL, (g + 1) * NHL)
        rows = []
        for dd0, dd1 in ((0, 8), (8, 16)):
            for tn in range(2):
                for h in hs:
                    rows.append(w3[tn, h, dd0:dd1])         # [8, C]
        for tn in range(2):
            for h in hs:
                rows.append(w3[tn, h, 16:64])               # [48, C]
        wqk = np.concatenate(rows, axis=0)                  # [768, C]
        wqkt = np.ascontiguousarray(wqk.T).astype(BF)       # [C, 768]
        wv = w3[2, list(hs)].reshape(CL, C)                 # [384, C]
        wvt = np.ascontiguousarray(wv.T).astype(BF)
        wotr = np.ascontiguousarray(wo[:, g * CL:(g + 1) * CL].T).astype(BF)
        per_group.append((wqkt, wvt, wotr))

    j = np.arange(RD // 2, dtype=np.float64)
    freqs = 1.0 / (10000.0 ** (2 * j / RD))
    t = np.arange(T, dtype=np.float64)
    ang = t[None, :] * freqs[:, None]                        # [8, T]
    cosb = np.ascontiguousarray(np.tile(np.cos(ang), (12, 1))).astype(BF)
    sinb = np.ascontiguousarray(np.tile(np.sin(ang), (12, 1))).astype(BF)

    kk = np.arange(128)[:, None]
    qq = np.arange(128)[None, :]
    tri1 = (kk <= qq).astype(BF)
    tri = np.concatenate([tri1, tri1], axis=1)  # [128, 256], one per head
    e6 = np.zeros((6, NPAIR * 128), dtype=np.float32)
    for p in range(NPAIR):
        e6[2 * p, p * 128:p * 128 + 64] = 1.0
        e6[2 * p + 1, p * 128 + 64:(p + 1) * 128] = 1.0
    e6 = e6.astype(BF)

    in_maps = []
    for i in range(8):
        b, g = divmod(i, 2)
        wqkt, wvt, wotr = per_group[g]
        in_maps.append({
            "xt": np.ascontiguousarray(xf[b].T).astype(BF),
            "wqkt": wqkt, "wvt": wvt, "wot": wotr,
            "cosb": cosb, "sinb": sinb, "tri": tri, "e6": e6,
        })
    return in_maps


def kernel(x, w_qkv, w_out, _trace=False):
    from concourse.bass_utils import run_bass_kernel_spmd

    if "nc" not in _cache:
        _cache["nc"] = _build()
    nc = _cache["nc"]
    in_maps = _host_inputs(x, w_qkv, w_out)
    res = run_bass_kernel_spmd(nc, in_maps, core_ids=list(range(8)),
                               trace=_trace)
    _cache["last_result"] = res
    out = np.empty((B, T, C), dtype=np.float32)
    for b in range(B):
        acc = res.results[2 * b]["out"].astype(np.float32) + \
            res.results[2 * b + 1]["out"].astype(np.float32)
        out[b] = acc.T
    return out


# revision 50
# speedup vs baseline: 1.0044x; 1.0044x over previous
"""Causal multi-head attention block (qkv proj + partial RoPE + causal attn +
out proj) for Trainium2, distributed over 8 NeuronCores.

Sharding: core i handles batch b = i//2 and head-group g = i%2 (6 of 12 heads).
Each core computes a partial output projection (contraction over its 6 heads'
384 channels); the host sums the two head-group partials per batch.

v9 design notes (v2 342us, v4 300us, v5 294us, v6 287us, v8 285us; v9
measured 319us vs v8's 336us under the same ~18% P0 thermal downclock,
i.e. ~270us at full clock -- engine-busy ratios v9/v8 are uniformly ~1.19,
confirming identical instruction mix at a lower clock with better overlap):
  - All-bf16 matmul paths. exp is the only Scalar-engine op.
  - Attention inner loop software-pipelined by one k-tile: the score pair +
    exp of kt+1 are emitted BEFORE the attn@v matmuls of kt, so the strict
    in-order PE queue never parks ready score work behind an av matmul
    that is waiting on its exp.
  - Pipeline: projection of tile jt+2 is EMITTED two iterations ahead
    (projections depend only on external inputs), so the projection chain
    (x DMA -> rope matmuls -> DVE rope -> scatter DMAs -> pass evictions)
    has a full iteration of slack: nothing in it can gate the exp stream
    or the attention matmuls, and the DVE eviction queue order stops
    mattering.  v5 measured the tile-0 scatter stream alone at ~15us.
  - Causal mask applied in one GpSimd op per diagonal k-tile over both
    heads.  (A 24->4 merged rope scatter via partition-split DMA views
    produced wrong data -- reverted to 24 small DMAs split across rings.)
  - Iteration emit order: attention(jt) | projection(jt+2) | finalize(jt-1)
    (finalize = softmax-normalize + out-projection, software-pipelined one
    tile behind; its serial chain fills engine gaps).
  - Scatter/bulk DMAs split across both HWDGE rings (sync + scalar) to
    double the small-transfer stream rate.
  - PSUM: sg scores [128,2,512]x2 (4 banks) + o accumulator x1 (2 banks) +
    merged flex pool [128,512]x2 (2 banks) shared by rope/pass/v projection
    tiles, broadcast, and out-projection.
  - Softmax: no max-subtraction needed (|scores/8| < ~3), denominator via a
    ones-column in v (o row 64); o evicted bf16 by DVE, rowsums DMA'd to
    rs6, DVE reciprocal, K=6 broadcast matmul, DVE multiply into bf16 o_sb.
"""

import numpy as np

B, T, C = 4, 2048, 768
NH, HD, RD = 12, 64, 16
NHL = NH // 2          # heads per core (local)
NPAIR = NHL // 2       # head pairs per core
CL = NHL * HD          # local channels (384)
TQ = 512               # q tile
NTQ = T // TQ
NKT = T // 128         # k tiles of 128

_cache = {}


def _build(debug=False):
    import concourse.bacc as bacc
    import concourse.mybir as mybir
    import concourse.tile as tile

    F32 = mybir.dt.float32
    BF16 = mybir.dt.bfloat16
    AF = mybir.ActivationFunctionType
    MUL = mybir.AluOpType.mult
    SUB = mybir.AluOpType.subtract
    ADD = mybir.AluOpType.add

    nc = bacc.Bacc(trn_type="TRN2", name="attn8")

    xt = nc.dram_tensor("xt", [C, T], BF16, kind="ExternalInput")
    wqkt = nc.dram_tensor("wqkt", [C, 2 * CL], BF16, kind="ExternalInput")
    wvt = nc.dram_tensor("wvt", [C, CL], BF16, kind="ExternalInput")
    wot = nc.dram_tensor("wot", [CL, C], BF16, kind="ExternalInput")
    cosb = nc.dram_tensor("cosb", [96, T], BF16, kind="ExternalInput")
    sinb = nc.dram_tensor("sinb", [96, T], BF16, kind="ExternalInput")
    tri = nc.dram_tensor("tri", [128, 256], BF16, kind="ExternalInput")
    e6 = nc.dram_tensor("e6", [6, NPAIR * 128], BF16, kind="ExternalInput")
    out = nc.dram_tensor("out", [C, T], BF16, kind="ExternalOutput")

    # qk-projection M-tiles (wqkt column order, host-built):
    #   tile 0: r1 rows [96] = (q h0..h5 | k h0..h5) x dims 0:8
    #   tile 1: r2 rows [96] = same x dims 8:16
    #   tiles 2..6: pass [128,128,128,128,64] = (q h0..h5 | k h0..h5) x dims 16:64
    MT_SIZES = [96, 96, 128, 128, 128, 128, 64]
    MT_OFF = np.cumsum([0] + MT_SIZES).tolist()

    def pass_dest(row):
        a, r = divmod(row, 48)        # a: tensor-head 0..11 (q first), r: dim-16
        tn, hl = divmod(a, NHL)
        blk = (0 if tn == 0 else NPAIR) + hl // 2
        part = 64 * (hl % 2) + 16 + r
        return blk, part

    with tile.TileContext(nc) as tc:
        with (
            tc.tile_pool(name="persist", bufs=1) as pp,
            tc.tile_pool(name="weights", bufs=1) as wp,
            tc.tile_pool(name="xload", bufs=2) as xlp,
            tc.tile_pool(name="pstage", bufs=2) as psg,
            tc.tile_pool(name="ropet", bufs=2) as rtp,
            tc.tile_pool(name="expp", bufs=4) as xpp,
            tc.tile_pool(name="misc", bufs=2) as msc,
            tc.tile_pool(name="onorm", bufs=7) as onp,
            tc.tile_pool(name="rsix", bufs=2) as rsp,
            tc.tile_pool(name="flex", bufs=2, space="PSUM") as flx,
            tc.tile_pool(name="sps", bufs=2, space="PSUM") as sps,
            tc.tile_pool(name="ops", bufs=1, space="PSUM") as ops,
        ):
            qk_sb = pp.tile([128, 2 * NPAIR, T], BF16, tag="qk")
            v_sb = pp.tile([128, NKT, NHL, HD + 1], BF16, tag="v")
            o_sb = pp.tile([128, NPAIR, T], BF16, tag="o")
            cos_t = pp.tile([96, T], BF16, tag="cos")
            sin_t = pp.tile([96, T], BF16, tag="sin")
            tri_t = pp.tile([128, 256], BF16, tag="tri")
            e6_t = pp.tile([6, NPAIR * 128], BF16, tag="e6")
            rot1 = pp.tile([96, T], BF16, tag="rot1")
            rot2 = pp.tile([96, T], BF16, tag="rot2")

            wqk_t = wp.tile([128, C // 128, 2 * CL], BF16, tag="wqk")
            wv_t = wp.tile([128, C // 128, CL], BF16, tag="wv")
            wo_t = wp.tile([128, NPAIR, C], BF16, tag="wo")

            # sync ring: wqk/wv/x + half the scatters (the critical path).
            # scalar ring: cos/sin/wo + other half + rs6 + out stores.
            nc.sync.dma_start(wqk_t, wqkt.rearrange("(co p) m -> p co m", p=128))
            nc.sync.dma_start(tri_t, tri[:, :])
            nc.sync.dma_start(e6_t, e6[:, :])
            nc.gpsimd.memset(
                v_sb[:, :, :, HD:HD + 1]
                .bitcast(mybir.dt.uint16)
                .rearrange("p a b c -> p (a b c)"),
                0x3F80)  # bf16 1.0 bit pattern
            # x(0) first on the scalar ring: the first qk matmuls need only
            # wqk (sync ring) + x(0); cos/sin/wv/wo are consumed later.
            x_pre = xlp.tile([128, C // 128, TQ], BF16, tag="x")
            nc.scalar.dma_start(
                x_pre, xt.rearrange("(co p) t -> p co t", p=128)[:, :, 0:TQ])
            nc.scalar.dma_start(cos_t, cosb[:, :])
            nc.scalar.dma_start(sin_t, sinb[:, :])
            nc.scalar.dma_start(wv_t, wvt.rearrange("(co p) m -> p co m", p=128))
            nc.scalar.dma_start(wo_t, wot.rearrange("(po p) m -> p po m", p=128))

            def _project(jt):
                ts = slice(jt * TQ, (jt + 1) * TQ)
                if jt == 0:
                    x_jt = x_pre
                else:
                    x_jt = xlp.tile([128, C // 128, TQ], BF16, tag="x")
                    nc.scalar.dma_start(
                        x_jt,
                        xt.rearrange("(co p) t -> p co t", p=128)[:, :, ts])

                psa = flx.tile([128, TQ], F32, tag="flex")
                psb = flx.tile([128, TQ], F32, tag="flex")
                for mt, psr in ((0, psa), (1, psb)):
                    R = MT_SIZES[mt]
                    for c in range(C // 128):
                        nc.tensor.matmul(
                            psr[0:R], wqk_t[:, c, MT_OFF[mt]:MT_OFF[mt + 1]],
                            x_jt[:, c], start=(c == 0), stop=(c == C // 128 - 1))
                # rope: rot1 = r1*cos - r2*sin ; rot2 = r2*cos + r1*sin
                t1 = rtp.tile([96, TQ], F32, tag="t1")
                t2 = rtp.tile([96, TQ], F32, tag="t2")
                nc.vector.tensor_tensor(t1, psa[0:96], cos_t[:, ts], MUL)
                nc.vector.tensor_tensor(t2, psb[0:96], sin_t[:, ts], MUL)
                nc.vector.tensor_tensor(rot1[:, ts], t1, t2, SUB)
                t3 = rtp.tile([96, TQ], F32, tag="t1")
                t4 = rtp.tile([96, TQ], F32, tag="t2")
                nc.vector.tensor_tensor(t3, psb[0:96], cos_t[:, ts], MUL)
                nc.vector.tensor_tensor(t4, psa[0:96], sin_t[:, ts], MUL)
                nc.vector.tensor_tensor(rot2[:, ts], t3, t4, ADD)
                # scatters 3-way split: both HWDGE rings + the gpsimd SWDGE
                # path as a third independent stream (~1 DMA/us each)
                rings = (nc.sync, nc.scalar, nc.gpsimd)
                for a in range(12):
                    tn, hl = divmod(a, NHL)
                    blk = (0 if tn == 0 else NPAIR) + hl // 2
                    base = 64 * (hl % 2)
                    eng = rings[a % 3]
                    eng.dma_start(qk_sb[base:base + 8, blk, ts],
                                  rot1[8 * a:8 * a + 8, ts])
                    eng.dma_start(qk_sb[base + 8:base + 16, blk, ts],
                                  rot2[8 * a:8 * a + 8, ts])

                nsc = 0
                for mt in range(2, 7):
                    R = MT_SIZES[mt]
                    ps = flx.tile([128, TQ], F32, tag="flex")
                    for c in range(C // 128):
                        nc.tensor.matmul(
                            ps[0:R], wqk_t[:, c, MT_OFF[mt]:MT_OFF[mt + 1]],
                            x_jt[:, c], start=(c == 0), stop=(c == C // 128 - 1))
                    stg = psg.tile([128, TQ], BF16, tag="pstg")
                    nc.vector.tensor_copy(stg[0:R], ps[0:R])
                    row0 = MT_OFF[mt] - MT_OFF[2]
                    row = row0
                    while row < row0 + R:
                        blk, part = pass_dest(row)
                        run = min(row0 + R - row, 48 - row % 48)
                        eng = rings[nsc % 3]
                        nsc += 1
                        eng.dma_start(
                            qk_sb[part:part + run, blk, ts],
                            stg[row - row0:row - row0 + run])
                        row += run

                for vt in range(TQ // 128):
                    pvf = flx.tile([128, TQ], F32, tag="flex")
                    pv = pvf[:, 0:CL]
                    kt0 = jt * (TQ // 128) + vt
                    for c in range(C // 128):
                        nc.tensor.matmul(
                            pv, x_jt[:, c, vt * 128:(vt + 1) * 128],
                            wv_t[:, c], start=(c == 0), stop=(c == C // 128 - 1))
                    nc.vector.tensor_copy(
                        v_sb[:, kt0, :, 0:HD],
                        pv.rearrange("p (h d) -> p h d", d=HD))

            def _attention(jq):
                qs = slice(jq * TQ, (jq + 1) * TQ)
                ouns = []
                rs6_t = rsp.tile([6, TQ], BF16, tag="rs6")
                for p in range(NPAIR):
                    qb = qk_sb[:, p, qs]
                    kb = qk_sb[:, NPAIR + p, :]
                    o_ps = ops.tile([128, 2, TQ], F32, tag="o")
                    nkt = 4 * (jq + 1)
                    # scores/exp emitted one k-tile ahead of the attn@v
                    # matmuls: the PE queue is strict in-order, so putting
                    # av(kt) (which waits on exp(kt)) ahead of the ready
                    # score pair (kt+1) stalls the ready work behind it.
                    pend_av = None

                    def _av(item, last):
                        kt, a = item
                        for h in range(2):
                            nc.tensor.matmul(
                                o_ps[0:65, h, a:TQ],
                                v_sb[:, kt, 2 * p + h, :],
                                eps[kt][:, h, a:TQ],
                                start=(kt == 0), stop=last)

                    eps = {}
                    for kt in range(nkt):
                        m = kt - 4 * jq
                        a = 0 if m < 0 else 128 * m
                        ks = slice(kt * 128, (kt + 1) * 128)
                        sg = sps.tile([128, 2, TQ], F32, tag="s")
                        nc.tensor.matmul(
                            sg[:, 0, a:TQ], kb[0:64, ks], qb[0:64, a:TQ],
                            start=True, stop=True, tile_position=(0, 0))
                        nc.tensor.matmul(
                            sg[:, 1, a:TQ], kb[64:128, ks], qb[64:128, a:TQ],
                            start=True, stop=True, tile_position=(64, 0))
                        ep = xpp.tile([128, 2, TQ], BF16, tag="e")
                        nc.scalar.activation(ep[:, :, a:TQ], sg[:, :, a:TQ],
                                             AF.Exp, scale=0.125)
                        if m >= 0:
                            nc.gpsimd.tensor_tensor(
                                ep[:, :, a:a + 128],
                                ep[:, :, a:a + 128],
                                tri_t.rearrange("p (h w) -> p h w", h=2), MUL)
                        eps[kt] = ep
                        if pend_av is not None:
                            _av(pend_av, last=False)
                            del eps[pend_av[0]]
                        pend_av = (kt, a)
                    _av(pend_av, last=True)
                    # o -> SBUF unnormalized (DVE, bf16); rowsum row -> rs6
                    oun = onp.tile([128, 2, TQ], BF16, tag="oun")
                    nc.vector.tensor_copy(oun[0:65, :, :], o_ps[0:65, :, :])
                    ouns.append(oun)
                    for h in range(2):
                        nc.gpsimd.dma_start(
                            rs6_t[2 * p + h:2 * p + h + 1, :],
                            oun[64:65, h, :])
                return (jq, qs, ouns, rs6_t)

            def _finalize(fjt, qs, ouns, rs6_t):
                # batched softmax denominators for all 3 pairs of this q-tile
                rinv6_t = rsp.tile([6, TQ], BF16, tag="rinv6")
                with nc.allow_low_precision(reason="softmax denom in bf16"):
                    nc.vector.reciprocal(rinv6_t, rs6_t)
                for p in range(NPAIR):
                    bc = flx.tile([128, TQ], F32, tag="flex")
                    nc.tensor.matmul(bc, e6_t[:, p * 128:(p + 1) * 128],
                                     rinv6_t, start=True, stop=True)
                    oun = ouns[p]
                    nc.vector.tensor_tensor(
                        o_sb[0:64, p, qs], oun[0:64, 0, :], bc[0:64], MUL)
                    nc.vector.tensor_tensor(
                        o_sb[64:128, p, qs], oun[0:64, 1, :], bc[64:128], MUL)
                # output projection for this t-tile
                for dt in range(C // 128):
                    po = flx.tile([128, TQ], F32, tag="flex")
                    for p in range(NPAIR):
                        nc.tensor.matmul(
                            po, wo_t[:, p, dt * 128:(dt + 1) * 128],
                            o_sb[:, p, qs], start=(p == 0), stop=(p == NPAIR - 1))
                    ost = msc.tile([128, TQ], BF16, tag="ost")
                    nc.vector.tensor_copy(ost, po)
                    nc.scalar.dma_start(
                        out.rearrange("(do p) t -> do p t", p=128)[dt, :, qs], ost)

            _project(0)
            pending = []
            for jt in range(NTQ):
                pending.append(_attention(jt))
                if jt == 0:
                    _project(1)   # after att(0) so attention outranks it
                if jt + 2 < NTQ:
                    _project(jt + 2)
                if jt > 0:
                    _finalize(*pending.pop(0))
            _finalize(*pending.pop(0))

    nc.compile()
    return nc


def _host_inputs(x, w_qkv, w_out):
    """Build per-core input dicts. Core i: batch i//2, head-group i%2."""
    import ml_dtypes

    BF = ml_dtypes.bfloat16
    xf = np.ascontiguousarray(x, dtype=np.float32)
    w3 = np.asarray(w_qkv, dtype=np.float32).reshape(3, NH, HD, C)
    wo = np.asarray(w_out, dtype=np.float32)

    per_group = []
    for g in range(2):
        hs = range(g * NHL, (g + 1) * NHL)
        rows = []
        for dd0, dd1 in ((0, 8), (8, 16)):
            for tn in range(2):
                for h in hs:
                    rows.append(w3[tn, h, dd0:dd1])         # [8, C]
        for tn in range(2):
            for h in hs:
                rows.append(w3[tn, h, 16:64])               # [48, C]
        wqk = np.concatenate(rows, axis=0)                  # [768, C]
        wqkt = np.ascontiguousarray(wqk.T).astype(BF)       # [C, 768]
        wv = w3[2, list(hs)].reshape(CL, C)                 # [384, C]
        wvt = np.ascontiguousarray(wv.T).astype(BF)
        wotr = np.ascontiguousarray(wo[:, g * CL:(g + 1) * CL].T).astype(BF)
        per_group.append((wqkt, wvt, wotr))

    j = np.arange(RD // 2, dtype=np.float64)
    freqs = 1.0 / (10000.0 ** (2 * j / RD))
    t = np.arange(T, dtype=np.float64)
    ang = t[None, :] * freqs[:, None]                        # [8, T]
    cosb = np.ascontiguousarray(np.tile(np.cos(ang), (12, 1))).astype(BF)
    sinb = np.ascontiguousarray(np.tile(np.sin(ang), (12, 1))).astype(BF)

    kk = np.arange(128)[:, None]
    qq = np.arange(128)[None, :]
    tri1 = (kk <= qq).astype(BF)
    tri = np.concatenate([tri1, tri1], axis=1)  # [128, 256], one per head
    e6 = np.zeros((6, NPAIR * 128), dtype=np.float32)
    for p in range(NPAIR):
        e6[2 * p, p * 128:p * 128 + 64] = 1.0
        e6[2 * p + 1, p * 128 + 64:(p + 1) * 128] = 1.0
    e6 = e6.astype(BF)

    in_maps = []
    for i in range(8):
        b, g = divmod(i, 2)
        wqkt, wvt, wotr = per_group[g]
        in_maps.append({
            "xt": np.ascontiguousarray(xf[b].T).astype(BF),
            "wqkt": wqkt, "wvt": wvt, "wot": wotr,
            "cosb": cosb, "sinb": sinb, "tri": tri, "e6": e6,
        })
    return in_maps


def kernel(x, w_qkv, w_out, _trace=False):
    from concourse.bass_utils import run_bass_kernel_spmd

    if "nc" not in _cache:
        _cache["nc"] = _build()
    nc = _cache["nc"]
    in_maps = _host_inputs(x, w_qkv, w_out)
    res = run_bass_kernel_spmd(nc, in_maps, core_ids=list(range(8)),
                               trace=_trace)
    _cache["last_result"] = res
    out = np.empty((B, T, C), dtype=np.float32)
    for b in range(B):
        acc = res.results[2 * b]["out"].astype(np.float32) + \
            res.results[2 * b + 1]["out"].astype(np.float32)
        out[b] = acc.T
    return out
